# revision 11
# baseline (speedup 1.0000x reference)
"""Trainium2 Bass kernel for nn_Attention (B=4, C=256, L=2048, H=8 heads, D=64).

Sharding: head-parallel across 8 NeuronCores (1 head per core). Each core:
  - projects its head's Q/K/V from the full input x (channels-first, fp16),
  - runs attention in the S^T (keys-on-partitions) layout so softmax's
    denominator comes free from an appended ones-column in the V^T lhsT
    (M=65 matmul),
  - normalizes + casts its head output to fp16,
  - per half-batch AllToAlls redistribute head outputs: each core k owns
    64-column granules g with g % 8 == k of every batch (granule mapping:
    l = ib*512 + k*64 + t for shard column s = ib*64 + t),
  - each core applies w_out + bias on its column shard.
Host reassembles the 8 granule-interleaved shards into the full [B, C, L]
output.

The attention inner loop is ACT(Exp)-bound (~1.1us per jp step); the PE is
kept continuously busy (so it ramps to its 2.4 GHz p-state) by interleaving
"filler" work — the next batch's QKV projection and the previous batch's
output projection — one small chunk per jp step between the S and PV
matmuls.
"""

import os
import sys

import numpy as np

sys.path.insert(0, "/opt/trn_rl_repo")

import concourse.bass as bass  # noqa: E402
import concourse.bacc as bacc  # noqa: E402
import concourse.tile as tile  # noqa: E402
import concourse.mybir as mybir  # noqa: E402
import concourse.bass_utils as bass_utils  # noqa: E402
from concourse.bass_interp import get_hw_module  # noqa: E402

B, C, L = 4, 256, 2048
H, D = 8, 64
NCORES = 8
N = B * L                # 8192 flattened (b, l) columns
LSH = L // NCORES        # 256 l-columns per core in the output shard
NBLK = 512               # matmul free-dim block
F32 = mybir.dt.float32
F16 = mybir.dt.float16
AF = mybir.ActivationFunctionType

_CACHE = {}


def _build():
    nc = bacc.Bacc("TRN2", target_bir_lowering=False, debug=False,
                   num_devices=NCORES)

    x_t = nc.dram_tensor("x_t", [2, 128, N], F16, kind="ExternalInput")
    # [c_lo, ch, (q|k) out] merged Q+K projection weights
    wqk_p = nc.dram_tensor("wqk_p", [128, 2, 128], F16, kind="ExternalInput")
    wv_p = nc.dram_tensor("wv_p", [128, 128], F16, kind="ExternalInput")
    wo_p = nc.dram_tensor("wo_p", [128, 4, 256], F16, kind="ExternalInput")
    bias2 = nc.dram_tensor("bias2", [128, 2], F32, kind="ExternalInput")
    out = nc.dram_tensor("out", [B, 2, 128, LSH], F32, kind="ExternalOutput")

    with tile.TileContext(nc) as tc:
        with (
            tc.tile_pool(name="const", bufs=1) as cpool,
            tc.tile_pool(name="qk", bufs=2) as qkpool,
            tc.tile_pool(name="vt", bufs=2) as vtpool,
            tc.tile_pool(name="pt", bufs=4) as ptpool,
            tc.tile_pool(name="small", bufs=4) as spool,
            tc.tile_pool(name="gh", bufs=2) as ghpool,
            tc.tile_pool(name="psA", bufs=2, space="PSUM") as psA,
            tc.tile_pool(name="psO", bufs=2, space="PSUM") as psO,
            tc.tile_pool(name="psP", bufs=2, space="PSUM") as psP,
            tc.tile_pool(name="dram", bufs=1, space="DRAM") as dpool,
        ):
            # ---- constants / weights into SBUF (weights first: tiny and
            # needed by the first projection; batch-0 x chunks next so the
            # first matmuls can start ASAP; wo/bias are only needed later) ----
            wqk_sb = cpool.tile([128, 256], F16, name="wqk_sb")
            wv_sb = cpool.tile([128, 128], F16, name="wv_sb")
            wo_sb = cpool.tile([128, 1024], F16, name="wo_sb")
            bias_sb = cpool.tile([128, 2], F32, name="bias_sb")
            nc.sync.dma_start(wqk_sb.rearrange("p (c o) -> p c o", c=2), wqk_p[:])
            nc.sync.dma_start(wv_sb[:], wv_p[:])
            x_sb = cpool.tile([128, 2 * N], F16, name="x_sb")
            for s in range(2):          # batch-0 columns first
                for ch in range(2):
                    nc.sync.dma_start(
                        x_sb[:, ch * N + s * 1024:ch * N + (s + 1) * 1024],
                        x_t[ch, :, s * 1024:(s + 1) * 1024],
                    )
            nc.sync.dma_start(wo_sb.rearrange("p (c o) -> p c o", c=4), wo_p[:])
            nc.sync.dma_start(bias_sb[:], bias2[:])
            for s in range(2, 8):
                for ch in range(2):
                    nc.sync.dma_start(
                        x_sb[:, ch * N + s * 1024:ch * N + (s + 1) * 1024],
                        x_t[ch, :, s * 1024:(s + 1) * 1024],
                    )

            # half-batch a2a staging: [half, core, d, 128 shard-cols]
            bnc_in = [dpool.tile([2, NCORES, 64, 128], F16, name=f"bnc_in{b}",
                                 tag=f"bnc_in{b}")
                      for b in range(B)]
            bnc_out = [dpool.tile([2, NCORES, 64, 128], F16, name=f"bnc_out{b}",
                                  tag=f"bnc_out{b}")
                       for b in range(B)]

            qd = {}
            kd = {}
            vt3 = {}

            # ---------------- filler emission (PE keep-warm) ----------------
            # Small chunks of independent work (next batch's projection,
            # previous batch's output projection) interleaved one per slot
            # into the ACT-bound attention inner loop.
            fillers = []

            def drain_fillers():
                while fillers:
                    fillers.pop(0)()

            def queue_projvt(b):
                """Queue batch b's QKV projection + V^T build as fillers."""
                st = {}

                def alloc():
                    qd[b] = qkpool.tile([128, L], F16, name="qd", tag="qd")
                    kd[b] = qkpool.tile([128, L], F16, name="kd", tag="kd")
                    vt3[b] = vtpool.tile(
                        [128, 16 * 65], F16, name="vt", tag="vt"
                    ).rearrange("p (j e) -> p j e", e=65)
                    nc.vector.memset(vt3[b][:, :, 64], 1.0)

                fillers.append(alloc)

                def qk_mm(nb, ch):
                    if ch == 0:
                        st['ps'] = psP.tile([128, NBLK], F32, name="psqk",
                                            tag="psp")
                    col0 = ch * N + b * L + nb * NBLK
                    nc.tensor.matmul(
                        st['ps'][:], wqk_sb[:, ch * 128:(ch + 1) * 128],
                        x_sb[:, col0:col0 + NBLK],
                        start=(ch == 0), stop=(ch == 1))

                def qk_copy(nb):
                    ps = st['ps']
                    nc.vector.tensor_copy(
                        qd[b][0:64, nb * NBLK:(nb + 1) * NBLK], ps[0:64, :])
                    nc.vector.tensor_copy(
                        kd[b][0:64, nb * NBLK:(nb + 1) * NBLK], ps[64:128, :])

                for nb in range(4):
                    fillers.append(lambda nb=nb: qk_mm(nb, 0))
                    fillers.append(lambda nb=nb: qk_mm(nb, 1))
                    fillers.append(lambda nb=nb: qk_copy(nb))
                # duplicate into the upper partition halves (for the
                # alternating tile_position S matmuls)
                fillers.append(lambda: nc.vector.tensor_copy(
                    qd[b][64:128, :], qd[b][0:64, :]))
                fillers.append(lambda: nc.vector.tensor_copy(
                    kd[b][64:128, :], kd[b][0:64, :]))

                # V^T directly: out[l, d] = sum_c x[c, l] * wv[c, d]
                def vt_mm(jp, half):
                    if half == 0:
                        st['pst'] = psP.tile([128, 128], F32, name="pst",
                                             tag="psp")
                    jt = 2 * jp + half
                    for ch in range(2):
                        col0 = ch * N + b * L + jt * 128
                        nc.tensor.matmul(
                            st['pst'][:, half * 64:(half + 1) * 64],
                            x_sb[:, col0:col0 + 128],
                            wv_sb[:, ch * 64:(ch + 1) * 64],
                            start=(ch == 0), stop=(ch == 1))

                def vt_copy(jp):
                    nc.vector.tensor_copy(
                        vt3[b][:, 2 * jp:2 * jp + 2, 0:64],
                        st['pst'].rearrange("p (j e) -> p j e", e=64))

                for jp in range(8):
                    fillers.append(lambda jp=jp: vt_mm(jp, 0))
                    fillers.append(lambda jp=jp: vt_mm(jp, 1))
                    fillers.append(lambda jp=jp: vt_copy(jp))

            def queue_yproj(b, h):
                """Queue output projection of batch b, half h (after the
                (b, h) a2a)."""
                st = {}

                def gather():
                    st['gh'] = ghpool.tile([128, 4, 128], F16, name="gh",
                                           tag=f"gh{h}")
                    for hc in range(4):
                        for hp in range(2):
                            nc.sync.dma_start(
                                st['gh'][hp * 64:(hp + 1) * 64, hc, :],
                                bnc_out[b][h, hc * 2 + hp, :, :])

                fillers.append(gather)

                def y_mm(oh, cpair):
                    if cpair == 0:
                        st['psy'] = psP.tile([128, 128], F32, name="psy",
                                             tag="psp")
                    for c in (2 * cpair, 2 * cpair + 1):
                        nc.tensor.matmul(
                            st['psy'][:],
                            wo_sb[:, c * 256 + oh * 128:c * 256 + (oh + 1) * 128],
                            st['gh'][:, c, :],
                            start=(c == 0), stop=(c == 3))

                def y_out(oh):
                    y = spool.tile([128, 128], F32, name="y", tag="y")
                    nc.vector.tensor_scalar_add(y[:], st['psy'],
                                                bias_sb[:, oh:oh + 1])
                    nc.sync.dma_start(out[b, oh, :, h * 128:(h + 1) * 128], y[:])

                for oh in range(2):
                    fillers.append(lambda oh=oh: y_mm(oh, 0))
                    fillers.append(lambda oh=oh: y_mm(oh, 1))
                    fillers.append(lambda oh=oh: y_out(oh))

            # ---------------- attention ----------------
            def emit_attention_iblk(b, ib):
                """Software-pipelined S/Exp/PV: PV(jp-1) is emitted after
                S(jp) so the PE streams S(jp) while the ACT engine runs
                Exp(jp-1). One filler slot per S and per PV keeps the PE
                busy through the ACT-bound steady state."""
                pso = psO.tile([65, NBLK], F32, name="pso", tag="pso")
                pts = {}

                def emit_s(jp):
                    jA, jB = 2 * jp, 2 * jp + 1
                    pss = psA.tile([128, 2 * NBLK], F32, name="pss", tag="pss")
                    nc.tensor.matmul(
                        pss[:, 0:NBLK],
                        kd[b][0:64, jA * 128:(jA + 1) * 128],
                        qd[b][0:64, ib * NBLK:(ib + 1) * NBLK],
                        start=True, stop=True, tile_position=(0, 0))
                    nc.tensor.matmul(
                        pss[:, NBLK:2 * NBLK],
                        kd[b][64:128, jB * 128:(jB + 1) * 128],
                        qd[b][64:128, ib * NBLK:(ib + 1) * NBLK],
                        start=True, stop=True, tile_position=(64, 0))
                    pt = ptpool.tile([128, 2 * NBLK], F16, name="pt", tag="pt")
                    nc.scalar.activation(pt[:], pss[:], AF.Exp)
                    pts[jp] = pt

                def emit_pv(jp):
                    jA, jB = 2 * jp, 2 * jp + 1
                    pt = pts.pop(jp)
                    nc.tensor.matmul(
                        pso[:], vt3[b][:, jA, :], pt[:, 0:NBLK],
                        start=(jp == 0), stop=False)
                    nc.tensor.matmul(
                        pso[:], vt3[b][:, jB, :], pt[:, NBLK:2 * NBLK],
                        start=False, stop=(jp == 7))

                def filler():
                    if fillers:
                        fillers.pop(0)()

                emit_s(0)
                filler()
                for jp in range(1, 8):
                    emit_s(jp)
                    filler()
                    emit_pv(jp - 1)
                    filler()
                emit_pv(7)
                recip = spool.tile([1, NBLK], F32, name="recip", tag="recip")
                nc.vector.reciprocal(recip[:], pso[64:65, :])
                bc = spool.tile([64, NBLK], F32, name="bc", tag="bc")
                nc.gpsimd.partition_broadcast(bc[:], recip[:])
                on = spool.tile([64, NBLK], F16, name="on", tag="on")
                nc.vector.tensor_mul(on[:], pso[0:64, :], bc[:])
                # granule-interleaved shard mapping: column k*64+t of `on`
                # goes to core k's shard slot ib*64+t — one strided DMA
                # (partition dim stays first on the SBUF side; the DRAM dest
                # AP is rearranged to match).
                nc.sync.dma_start(
                    bnc_in[b][ib // 2, :, :, (ib % 2) * 64:(ib % 2) * 64 + 64]
                    .rearrange("k p t -> p k t"),
                    on.rearrange("p (k t) -> p k t", k=8))

            def emit_a2a(b, h):
                nc.gpsimd.collective_compute(
                    "AllToAll", mybir.AluOpType.bypass,
                    replica_groups=[list(range(NCORES))],
                    ins=[bnc_in[b][h].opt()], outs=[bnc_out[b][h].opt()])

            # ---------------- schedule ----------------
            # Batch 0's projection runs up front (no attention to hide it
            # under); later batches' projections and the output projections
            # ride the filler slots.
            queue_projvt(0)
            drain_fillers()
            for b in range(B):
                if b + 1 < B:
                    queue_projvt(b + 1)
                for ib in range(4):
                    if b > 0 and ib == 0:
                        queue_yproj(b - 1, 0)   # a2a(b-1,0) done mid-batch b-1
                    if b > 0 and ib == 2:
                        queue_yproj(b - 1, 1)   # a2a(b-1,1) done early batch b
                    emit_attention_iblk(b, ib)
                    if ib == 1:
                        emit_a2a(b, 0)
                drain_fillers()
                emit_a2a(b, 1)
            queue_yproj(B - 1, 0)
            queue_yproj(B - 1, 1)
            drain_fillers()

    nc.compile()
    if not os.environ.get("BASS_SIM"):
        nc.m = get_hw_module(nc.m)
    return nc


def _prep_in_maps(x, w_qkv, w_out, b_out):
    scale = float(D) ** -0.5
    x = np.asarray(x, np.float32)
    w_qkv = np.asarray(w_qkv, np.float32)
    w_out = np.asarray(w_out, np.float32)
    b_out = np.asarray(b_out, np.float32)

    x_in = np.ascontiguousarray(
        x.transpose(1, 0, 2).reshape(C, N).reshape(2, 128, N)).astype(np.float16)
    wq = w_qkv[0:512].reshape(H, D, C) * scale
    wk = w_qkv[512:1024].reshape(H, D, C)
    wv = w_qkv[1024:1536].reshape(H, D, C)

    wo_p = np.ascontiguousarray(
        w_out.T.reshape(4, 2, 64, 256).transpose(1, 2, 0, 3).reshape(128, 4, 256)
    ).astype(np.float16)
    bias2 = np.ascontiguousarray(b_out.reshape(2, 128).T)

    in_maps = []
    for h in range(NCORES):
        # [c, 128] per half with columns [q 64 | k 64] stacked -> M=128
        wqk = np.concatenate([wq[h].T, wk[h].T], axis=1)  # [256, 128]
        wqk_packed = np.ascontiguousarray(
            wqk.reshape(2, 128, 128).transpose(1, 0, 2)).astype(np.float16)
        wv_packed = np.ascontiguousarray(
            wv[h].T.reshape(2, 128, 64).transpose(1, 0, 2).reshape(128, 128)
        ).astype(np.float16)
        in_maps.append({
            "x_t": x_in,
            "wqk_p": wqk_packed,
            "wv_p": wv_packed,
            "wo_p": wo_p,
            "bias2": bias2,
        })
    return in_maps


def _run(inputs, trace=False):
    if "nc" not in _CACHE:
        _CACHE["nc"] = _build()
    nc = _CACHE["nc"]
    in_maps = _prep_in_maps(**inputs)
    res = bass_utils.run_bass_kernel_spmd(
        nc, in_maps, core_ids=list(range(NCORES)), trace=trace)
    # granule mapping: shard j's column s = ib*64 + t holds l = ib*512 +
    # j*64 + t
    shards = np.stack([res.results[j]["out"].reshape(B, C, LSH)
                       for j in range(NCORES)])      # [j, B, C, 4*64]
    y = shards.reshape(NCORES, B, C, 4, 64).transpose(1, 2, 3, 0, 4)
    y = np.ascontiguousarray(y.reshape(B, C, L), np.float32)
    return y, res


def kernel(x, w_qkv, w_out, b_out):
    y, _ = _run(dict(x=x, w_qkv=w_qkv, w_out=w_out, b_out=b_out), trace=False)
    return y
